# revision 36
# baseline (speedup 1.0000x reference)
"""Trainium2 Bass kernel for nn_CROM_Layer_81140522156285 (moe_routing).

Math restructure (exactly equivalent to the reference, far less work):
  last = x[:, -1, :]
  q    = last @ Wq.T
  qk   = (q @ Wk) / sqrt(D)              # tiny [B, D]
  scores[b, s] = x[b, s, :] . qk[b, :]   # one pass over x  (big, memory-bound)
  attn = softmax(scores)
  ctx  = (attn[b] @ x[b]) @ Wv.T
  out  = ctx @ expert_W[eid].T + expert_b[eid]
  y    = x with last row replaced by LayerNorm(last + out)

Device-side design (bf16, measured-cost driven):
  * x sent bf16, packed batch-INTERLEAVED: partition p serves batch p//32 for
    every tile, so one [128, D] qk tile works for all tiles and the ctx
    accumulation uses a single [4, 1024] PSUM group with a [128, 4]
    batch-masked stationary.
  * Score path per [128, D] tile, split to balance DVE vs ScalarE:
    'V' tiles: DVE fused scalar_tensor_tensor (mult+accum, 1217ns).
    'S' tiles: DVE all-bf16 2x multiply (685ns, batched 3D over runs) +
    ScalarE copy+accum reduce (1146+278ns).
    GpSimd is deliberately unused: concurrent Pool streaming contends for
    SBUF ports and halves DVE throughput (measured 685 -> 2114ns).
  * Variable chunk sizes [4,8,8,8,2,2]: small first chunk starts the
    reduce/exp/matmul pipeline early, small last chunks shrink the
    end-of-kernel matmul tail.
  * Tiny dummy matmuls spread across the reduce phase keep the PE HAM
    clock warm (otherwise every ctx matmul runs at 1.2 instead of 2.4 GHz).
  * Sequence dim split 1024-per-core across 8 cores (softmax partials
    combine linearly); host combines and applies the tiny tail projections.
"""

import numpy as np
import ml_dtypes

import concourse.bass as bass
import concourse.tile as tile
from concourse import bacc, mybir
from concourse.bass_utils import run_bass_kernel_spmd

B = 4
S = 8192
D = 1024
N_CORES = 8
S_CORE = S // N_CORES      # positions per batch handled by one core
P = 128                    # SBUF partitions
G = P // B                 # partitions per batch group (32)
T = (B * S_CORE) // P      # s-tiles of 128 interleaved positions per core (32)

BF16 = mybir.dt.bfloat16
F32 = mybir.dt.float32

# chunk sizes (tiles), per-chunk sub-DMA widths, per-chunk V/S assignment.
# 18 V (DVE-fused) / 14 S (DVE mult + ACT reduce) balances both engines.
CHUNKS = [2, 4, 8, 8, 8, 2]
SUB_W = [[1, 1], [2, 2], [4, 4], [4, 4], [4, 4], [2]]
ASSIGN_CHUNK = [
    ["V", "S"],
    ["V", "S", "S", "V"],
    ["V", "S", "S", "S", "S", "V", "S", "V"],
    ["V", "S", "S", "S", "S", "V", "S", "V"],
    ["V", "S", "S", "S", "S", "V", "S", "V"],
    ["V", "V"],
]

_NC = None


def _build_nc():
    nc = bacc.Bacc("TRN2", target_bir_lowering=False, debug=False,
                   num_devices=N_CORES)
    xs_ap = nc.dram_tensor("xs", [P, T, D], BF16, kind="ExternalInput").ap()
    # [:, 0:D] = qk row per batch group; [:, D] = 1.0 (z rhs); [:, D+1] pad;
    # [:, D+2 : D+2+B] = batch-mask columns (1.0 iff p//G == b)
    qkb_ap = nc.dram_tensor("qkb", [P, D + 2 + B], BF16,
                            kind="ExternalInput").ap()
    ctx_ap = nc.dram_tensor("ctx_out", [B, D], F32, kind="ExternalOutput").ap()
    z_ap = nc.dram_tensor("z_out", [B * 8, len(CHUNKS)], F32,
                          kind="ExternalOutput").ap()

    with tile.TileContext(nc) as tc:
        with (
            tc.tile_pool(name="const", bufs=1) as cpool,
            tc.tile_pool(name="x", bufs=5) as xpool,
            tc.tile_pool(name="prod", bufs=3) as ppool,
            tc.tile_pool(name="sc", bufs=6) as scpool,
            tc.tile_pool(name="psum", bufs=1, space="PSUM") as psumpool,
            tc.tile_pool(name="stg", bufs=1) as stgpool,
        ):
            qkb = cpool.tile([P, D + 2 + B], BF16, tag="qkb")
            xt0 = xpool.tile([P, 8, D], BF16, tag="xt")
            # first x tile before qkb: it is the longer pole at ramp
            nc.sync.dma_start(xt0[:, 0:1, :], xs_ap[:, 0:1, :])
            nc.sync.dma_start(qkb[:], qkb_ap[:])
            qk = qkb[:, 0:D]
            ones = qkb[:, D:D + 1]
            maskb = qkb[:, D + 2:D + 2 + B]

            # per-chunk esc tiles: [P, W, B] bf16 (w-major), batch-masked;
            # fully written by the mask multiply each chunk.
            escs = []
            escds = []
            for ci, W in enumerate(CHUNKS):
                esc_t = cpool.tile([P, W, B], BF16, tag=f"esc{ci}")
                escd_t = cpool.tile([P, W], BF16, tag=f"escd{ci}")
                escs.append(esc_t)
                escds.append(escd_t)

            ps_ctx = psumpool.tile([B, D], F32, tag="ctx")      # 2 banks
            # z per chunk: column ci of one [32, nchunk] bank; row = w*B + b
            # so batch = row % B regardless of chunk width
            ps_z = psumpool.tile([B * 8, len(CHUNKS)], F32, tag="z")
            ps_dum = psumpool.tile([1, 2], F32, tag="dum")      # HAM warmer

            dump_v = cpool.tile([P, D], BF16, tag="dump_v")
            dump_a = cpool.tile([P, D], BF16, tag="dump_a")

            n_p = max(sum(1 for a in ch if a == "S") for ch in ASSIGN_CHUNK)
            t0 = 0          # global tile index of chunk start
            for ci, W in enumerate(CHUNKS):
                assign = ASSIGN_CHUNK[ci]
                p_idx = {w: j for j, w in
                         enumerate(w for w in range(W) if assign[w] == "S")}
                xt = xt0 if ci == 0 else xpool.tile([P, 8, D], BF16, tag="xt")
                prod = ppool.tile([P, n_p, D], BF16, tag="prod")
                sc = scpool.tile([P, 8], F32, tag="sc")
                esc = escs[ci]

                off = 0
                for si, wsub in enumerate(SUB_W[ci]):
                    if not (ci == 0 and si == 0):   # chunk0/sub0 prefetched
                        xsl = xt[:, off:off + wsub, :]
                        nc.sync.dma_start(xsl, xs_ap[:, t0 + off:t0 + off + wsub, :])
                    w = off
                    while w < off + wsub:
                        if assign[w] == "V":
                            nc.vector.scalar_tensor_tensor(
                                out=dump_v[:], in0=xt[:, w, :], scalar=1.0,
                                in1=qk, op0=mybir.AluOpType.mult,
                                op1=mybir.AluOpType.mult,
                                accum_out=sc[:, w:w + 1])
                            w += 1
                        else:
                            # batch the 2x multiply over a consecutive S-run
                            w2 = w
                            while w2 < off + wsub and assign[w2] == "S":
                                w2 += 1
                            j = p_idx[w]
                            qk3 = qk.unsqueeze(1).broadcast_to([P, w2 - w, D])
                            nc.vector.tensor_tensor(
                                out=prod[:, j:j + (w2 - w), :],
                                in0=xt[:, w:w2, :], in1=qk3,
                                op=mybir.AluOpType.mult)
                            for wi in range(w, w2):
                                nc.scalar.activation(
                                    dump_a[:], prod[:, p_idx[wi], :],
                                    mybir.ActivationFunctionType.Copy,
                                    accum_out=sc[:, wi:wi + 1])
                            w = w2
                        if (t0 + w) % 2 == 0:
                            # tiny dummy matmul pinned to this tile's score:
                            # keeps the PE HAM activity window busy so the
                            # real ctx matmuls run at 2.4 GHz
                            nc.tensor.matmul(ps_dum[:, 0:1], sc[:, w - 1:w],
                                             sc[:, w - 1:w],
                                             start=True, stop=True)
                    off += wsub

                # dense exp (every partition's score is valid for its own
                # batch), then one tiny 2x multiply against the batch-mask
                # columns builds the [P, W, B] masked stationary
                escd = escds[ci]
                nc.scalar.activation(escd[:], sc[:, 0:W],
                                     mybir.ActivationFunctionType.Exp)
                nc.vector.tensor_tensor(
                    out=esc[:],
                    in0=escd[:].unsqueeze(2).broadcast_to([P, W, B]),
                    in1=maskb.unsqueeze(1).broadcast_to([P, W, B]),
                    op=mybir.AluOpType.mult)

                for w in range(W):
                    t = t0 + w
                    st, sp = (t == 0), (t == T - 1)
                    nc.tensor.matmul(ps_ctx[:, 0:512], esc[:, w, :],
                                     xt[:, w, 0:512], start=st, stop=sp)
                    nc.tensor.matmul(ps_ctx[:, 512:1024], esc[:, w, :],
                                     xt[:, w, 512:1024], start=st, stop=sp)
                # per-chunk z into its own ps_z column (w-major: row = w*B+b)
                nc.tensor.matmul(ps_z[0:B * W, ci:ci + 1],
                                 esc[:].rearrange("p a b -> p (a b)"),
                                 ones, start=True, stop=True)
                t0 += W

            stg = stgpool.tile([B, D], F32, tag="stg")
            stgz = stgpool.tile([B * 8, len(CHUNKS)], F32, tag="stgz")
            nc.vector.tensor_copy(stgz[:], ps_z[:])
            nc.sync.dma_start(z_ap[:], stgz[:])
            # split the PSUM->SBUF epilogue copy across ACT and DVE
            nc.scalar.activation(stg[:, 0:512], ps_ctx[:, 0:512],
                                 mybir.ActivationFunctionType.Copy)
            nc.vector.tensor_copy(stg[:, 512:1024], ps_ctx[:, 512:1024])
            nc.sync.dma_start(ctx_ap[:], stg[:])

    nc.compile()
    return nc


def _get_nc():
    global _NC
    if _NC is None:
        _NC = _build_nc()
    return _NC


def kernel(x_emb, Wq, Wk, Wv, expert_W, expert_b, ln_gamma, ln_beta,
           expert_id, _spmd_kwargs=None):
    x = np.ascontiguousarray(np.asarray(x_emb, dtype=np.float32))
    Wq = np.asarray(Wq, dtype=np.float32)
    Wk = np.asarray(Wk, dtype=np.float32)
    Wv = np.asarray(Wv, dtype=np.float32)
    expert_b = np.asarray(expert_b, dtype=np.float32)
    ln_gamma = np.asarray(ln_gamma, dtype=np.float32)
    ln_beta = np.asarray(ln_beta, dtype=np.float32)
    eid = int(np.asarray(expert_id))

    last = x[:, -1, :]                                   # [B, D]
    q = last @ Wq.T                                      # [B, D]
    qk = (q @ Wk) * np.float32(1.0 / np.sqrt(D))         # [B, D]

    qkb = np.zeros((P, D + 2 + B), dtype=ml_dtypes.bfloat16)
    qkb[:, 0:D] = np.repeat(qk, G, axis=0).astype(ml_dtypes.bfloat16)
    qkb[:, D] = ml_dtypes.bfloat16(1.0)
    for b in range(B):
        qkb[b * G:(b + 1) * G, D + 2 + b] = ml_dtypes.bfloat16(1.0)

    # per-core pack: [P, T, D] bf16 with partition p = batch p//G,
    # position within core shard = (p%G)*G + t  -> a single reshape
    in_maps = []
    for c in range(N_CORES):
        shard = x[:, c * S_CORE:(c + 1) * S_CORE, :]     # [B, S_CORE, D]
        xs = np.ascontiguousarray(
            shard.reshape(P, T, D).astype(ml_dtypes.bfloat16))
        in_maps.append({"xs": xs, "qkb": qkb})

    res = run_bass_kernel_spmd(_get_nc(), in_maps, core_ids=list(range(N_CORES)),
                               **(_spmd_kwargs or {}))
    ctx_un = np.zeros((B, D), dtype=np.float32)
    z = np.zeros((B, 1), dtype=np.float32)
    for c in range(N_CORES):
        ctx_un += res.results[c]["ctx_out"]
        zo = res.results[c]["z_out"]                     # [32, nchunk]
        for ci, W in enumerate(CHUNKS):
            blk = zo[0:B * W, ci].reshape(W, B)          # row = w*B + b
            z[:, 0] += blk.sum(axis=0)

    ctx = ctx_un / z                                     # [B, D] attn @ x
    context = ctx @ Wv.T                                 # [B, D]
    We = np.asarray(expert_W[eid], dtype=np.float32)     # [D, D]
    out = context @ We.T + expert_b[eid]                 # [B, D]
    resid = last + out
    mu = resid.mean(axis=-1, keepdims=True, dtype=np.float32)
    diff = resid - mu
    var = np.mean(diff * diff, axis=-1, keepdims=True, dtype=np.float32)
    new_focus = diff / np.sqrt(var + np.float32(1e-5)) * ln_gamma + ln_beta

    y = x.copy()
    y[:, -1, :] = new_focus
    return y


if __name__ == "__main__":
    rng = np.random.default_rng(0)
    xs = {
        "x_emb": rng.standard_normal((B, S, D), dtype=np.float32),
        "Wq": rng.standard_normal((D, D), dtype=np.float32) * 0.02,
        "Wk": rng.standard_normal((D, D), dtype=np.float32) * 0.02,
        "Wv": rng.standard_normal((D, D), dtype=np.float32) * 0.02,
        "expert_W": rng.standard_normal((128, D, D), dtype=np.float32) * 0.02,
        "expert_b": rng.standard_normal((128, D), dtype=np.float32) * 0.02,
        "ln_gamma": np.ones(D, dtype=np.float32),
        "ln_beta": np.zeros(D, dtype=np.float32),
        "expert_id": 7,
    }
    y = kernel(**xs)
    print(y.shape, y.dtype)
